# revision 17
# baseline (speedup 1.0000x reference)
"""Trainium2 Bass kernel for nn_BinaryLinear (binarized 4-layer MLP + BatchNorm).

Reference computation (fp32, jax):
    h = x.reshape(-1, 3072)
    h = relu(h @ sign(W1).T); h = BN(h, g1, b1)   # BN over full 8192 batch
    h = relu(h @ sign(W2).T); h = BN(h, g2, b2)
    h = relu(h @ sign(W3).T); h = BN(h, g3, b3)
    out = h @ sign(W4).T                          # [8192, 10]

Strategy (8 NeuronCores, data-parallel over batch):
  - Host: binarize weights to bf16 (+-1 exact), pack everything partition-
    major ([128, ktiles, free]), shard x over cores (1024 rows each).
  - Device (SPMD identical program): activations feature-major
    [feature_part, batch_free] in SBUF; K-tiled bf16 matmuls accumulate in
    fp32 PSUM. Per layer the 8 feature tiles run as groups [4,2,1,1]: the
    first four k-outer (consumes layer-1 input tiles in DMA arrival order),
    the rest sequential chains so relu/stats pipeline under the matmuls.
    The last two single-tile groups split relu/square per batch half so
    their stats leave before the chain tail retires.
  - Distributed BN: per-core (sum, sumsq) stats per group ride 4 small
    staggered AllGathers per layer; each group's gather + one-op core
    reduce + fused stats math + apply resolve under the following chains /
    the next layer's early k-tiles. A warmup AllGather absorbs the ~11us
    ncfw wake latency.
  - DMA-queue discipline: sync ring = XT feed, W2, all stats-out DMAs,
    final outputs; scalar ring = W1 feed, W3, W4, all gather DMAs; BNP on
    the GpSimd SWDGE queue. Queue contents are monotone in time, so no
    FIFO blocking at layer boundaries.
"""
import os
import sys

for _p in ("/opt/trn_rl_repo",):
    if os.path.isdir(_p) and _p not in sys.path:
        sys.path.insert(0, _p)

import numpy as np
import ml_dtypes

from concourse import bacc, tile, mybir
from concourse import bass_utils

NCORES = 8
B = 8192
BL = B // NCORES            # 1024 rows per core
HB = BL // 2                # 512-row batch half
KIN = 3072
KT_IN = KIN // 128          # 24 k-tiles for layer 1
HID = 1024
JT = HID // 128             # 8 feature tiles
CLS = 10
CLSP = 16                   # padded classes
EPS = 1e-5
BF16 = mybir.dt.bfloat16
F32 = mybir.dt.float32
ADD = mybir.AluOpType.add
SUB = mybir.AluOpType.subtract
MUL = mybir.AluOpType.mult
BYP = mybir.AluOpType.bypass
RELU = mybir.ActivationFunctionType.Relu
AXX = mybir.AxisListType.X
AXXY = mybir.AxisListType.XY

_CACHE = {}


def _build():
    nc = bacc.Bacc("TRN2", target_bir_lowering=False, debug=False, num_devices=NCORES)

    xt_d = nc.dram_tensor("xt", [128, KT_IN, BL], BF16, kind="ExternalInput")
    w1_d = nc.dram_tensor("w1t", [128, KT_IN, HID], BF16, kind="ExternalInput")
    w2_d = nc.dram_tensor("w2t", [128, JT, HID], BF16, kind="ExternalInput")
    w3_d = nc.dram_tensor("w3t", [128, JT, HID], BF16, kind="ExternalInput")
    w4_d = nc.dram_tensor("w4t", [128, JT, CLSP], BF16, kind="ExternalInput")
    bnp_d = nc.dram_tensor("bnp", [128, 6 * JT], F32, kind="ExternalInput")
    out_d = nc.dram_tensor("out", [CLSP, BL], F32, kind="ExternalOutput")

    nhalves = [(0, HB), (HB, HB)]
    bn_collectives = []  # BN stats AllGathers — trigger waits stripped below

    with tile.TileContext(nc) as tc:
        with (
            tc.tile_pool(name="weights", bufs=1) as wpool,
            tc.tile_pool(name="acts", bufs=1) as apool,
            tc.tile_pool(name="scratch", bufs=2) as scrpool,
            tc.tile_pool(name="stats", bufs=2) as spool,
            tc.tile_pool(name="psum", bufs=4, space="PSUM") as pspool,
            tc.tile_pool(name="dram", bufs=2, space="DRAM") as dpool,
        ):
            XT = wpool.tile([128, KT_IN, BL], BF16, tag="XT")
            W1 = wpool.tile([128, KT_IN, HID], BF16, tag="W1")
            W2 = wpool.tile([128, JT, HID], BF16, tag="W2")
            W3 = wpool.tile([128, JT, HID], BF16, tag="W3")
            W4 = wpool.tile([128, JT, CLSP], BF16, tag="W4")
            BNP = wpool.tile([128, 6 * JT], F32, tag="BNP")
            HRAW = apool.tile([128, JT, BL], BF16, tag="HRAW")
            H = apool.tile([128, JT, BL], BF16, tag="H")
            H2 = apool.tile([128, JT, BL], BF16, tag="H2")

            # Warmup collective: absorbs the ~11us ncfw wake latency off the
            # critical path. Input is an unwritten scratch buffer (contents
            # irrelevant); output anchored into an unused out_d row at the
            # end of the program so DCE keeps it.
            wu_in = dpool.tile([128, 1], F32, tag="wu_in")
            wu_out = dpool.tile([NCORES * 128, 1], F32, tag="wu_out")
            nc.gpsimd.collective_compute(
                "AllGather",
                BYP,
                replica_groups=[list(range(NCORES))],
                ins=[wu_in.opt()],
                outs=[wu_out.opt()],
            )

            # Input feed: XT on the Sync ring, W1 on the Scalar ring, in
            # growing chunks; W2/W3/W4 right after so nothing bulky is in
            # flight at the layer boundaries.
            nc.gpsimd.dma_start(BNP[:], bnp_d[:])
            feed = [2, 2, 4, 8, 8]
            c = 0
            for w in feed:
                w = min(w, KT_IN - c)
                if w <= 0:
                    break
                nc.sync.dma_start(XT[:, c : c + w, :], xt_d[:, c : c + w, :])
                nc.scalar.dma_start(W1[:, c : c + w, :], w1_d[:, c : c + w, :])
                c += w
            nc.sync.dma_start(W2[:], w2_d[:])
            nc.scalar.dma_start(W3[:], w3_d[:])
            nc.scalar.dma_start(W4[:], w4_d[:])

            def mm_pair(ps, Wk, rhs, k, kt):
                for idx, (s, w) in enumerate(nhalves):
                    mi = nc.tensor.matmul(
                        ps[:, s : s + w],
                        Wk,
                        rhs[:, k, s : s + w],
                        start=(k == 0),
                        stop=(k == kt - 1),
                    )
                    if idx > 0:
                        # same stationary weights as the previous matmul:
                        # skip the redundant LDWEIGHTS
                        mi.ins.ldweights = False

            def relu_square(ps, jt, S, j, n, split=False):
                """relu: PSUM f32 -> HRAW bf16, accum = batch sum (free).
                sum(relu^2) via one VectorE scalar_tensor_tensor. split=True
                accumulates per batch-half (S slots: sumA,sqA,sumB,sqB) so
                the stats leave before the whole chain retires."""
                if not split:
                    nc.scalar.activation(
                        HRAW[:, jt, :], ps[:], RELU, accum_out=S[:, j : j + 1]
                    )
                    scr = scrpool.tile([128, BL], BF16, tag="scr")
                    nc.vector.scalar_tensor_tensor(
                        scr[:], HRAW[:, jt, :], 0.0, HRAW[:, jt, :], BYP, MUL,
                        accum_out=S[:, n + j : n + j + 1],
                    )
                else:
                    for h, (s, w) in enumerate(nhalves):
                        nc.scalar.activation(
                            HRAW[:, jt, s : s + w], ps[:, s : s + w], RELU,
                            accum_out=S[:, 2 * h : 2 * h + 1],
                        )
                        scr = scrpool.tile([128, HB], BF16, tag="scrh")
                        nc.vector.scalar_tensor_tensor(
                            scr[:], HRAW[:, jt, s : s + w], 0.0,
                            HRAW[:, jt, s : s + w], BYP, MUL,
                            accum_out=S[:, 2 * h + 1 : 2 * h + 2],
                        )

            def bn_ag_start(li, gi, S, width):
                """Stats out on the Sync HWDGE ring + AllGather trigger."""
                cc_in = dpool.tile([128, width], F32, tag=f"cc_in{gi}",
                                   name=f"cc_in_{li}_{gi}")
                cc_out = dpool.tile([NCORES * 128, width], F32, tag=f"cc_out{gi}",
                                    name=f"cc_out_{li}_{gi}")
                nc.sync.dma_start(cc_in[:], S[:])
                cc = nc.gpsimd.collective_compute(
                    "AllGather",
                    BYP,
                    replica_groups=[list(range(NCORES))],
                    ins=[cc_in.opt()],
                    outs=[cc_out.opt()],
                )
                bn_collectives.append(cc)
                return cc_out

            def bn_finish(li, gi, jts, cc_out, Hdst, split=False):
                """Gather stats, one-op core (+half) reduce, fused math,
                apply into Hdst."""
                n = len(jts)
                width = 4 * n if split else 2 * n
                GAT = spool.tile([128, NCORES, width], F32, tag=f"GAT{gi}",
                                 name=f"GAT_{li}_{gi}")
                # sync ring, NOT scalar: a gather blocked on its AllGather in
                # the Scalar HWDGE FIFO stalls the next RELUs (~11us/boundary)
                nc.sync.dma_start(
                    GAT[:], cc_out.opt().rearrange("(c p) s -> p c s", p=128)
                )
                SS = spool.tile([128, 2 * n], F32, tag=f"SS{gi}",
                                name=f"SS_{li}_{gi}")
                if split:
                    nc.vector.tensor_reduce(
                        SS[:], GAT[:].rearrange("p c (h s) -> p s h c", h=2),
                        axis=AXXY, op=ADD,
                    )
                else:
                    nc.vector.tensor_reduce(
                        SS[:], GAT[:].rearrange("p c s -> p s c"),
                        axis=AXX, op=ADD,
                    )
                MV = spool.tile([128, 2 * n], F32, tag=f"MV{gi}",
                                name=f"MV_{li}_{gi}")
                nc.vector.tensor_scalar_mul(MV[:], SS[:], 1.0 / B)
                mean = MV[:, 0:n]
                esq = MV[:, n : 2 * n]
                MSQ = spool.tile([128, n], F32, tag=f"MSQ{gi}",
                                 name=f"MSQ_{li}_{gi}")
                nc.vector.scalar_tensor_tensor(MSQ[:], mean, 0.0, mean, BYP, MUL)
                VAR = spool.tile([128, n], F32, tag=f"VAR{gi}",
                                 name=f"VAR_{li}_{gi}")
                nc.vector.scalar_tensor_tensor(VAR[:], esq, EPS, MSQ[:], ADD, SUB)
                RINV = spool.tile([128, n], F32, tag=f"RINV{gi}",
                                  name=f"RINV_{li}_{gi}")
                nc.vector.reciprocal(RINV[:], VAR[:])
                RSTD = spool.tile([128, n], F32, tag=f"RSTD{gi}",
                                  name=f"RSTD_{li}_{gi}")
                nc.scalar.sqrt(RSTD[:], RINV[:])
                g0 = (2 * li) * JT + jts[0]
                b0 = (2 * li + 1) * JT + jts[0]
                A = spool.tile([128, n], F32, tag=f"A{gi}", name=f"A_{li}_{gi}")
                nc.vector.tensor_tensor(A[:], RSTD[:], BNP[:, g0 : g0 + n], MUL)
                AM = spool.tile([128, n], F32, tag=f"AM{gi}", name=f"AM_{li}_{gi}")
                nc.vector.tensor_tensor(AM[:], A[:], mean, MUL)
                C = spool.tile([128, n], F32, tag=f"C{gi}", name=f"C_{li}_{gi}")
                nc.vector.tensor_tensor(C[:], BNP[:, b0 : b0 + n], AM[:], SUB)
                for j, jt in enumerate(jts):
                    if split:
                        for s, w in nhalves:
                            nc.vector.tensor_scalar(
                                Hdst[:, jt, s : s + w], HRAW[:, jt, s : s + w],
                                A[:, j : j + 1], C[:, j : j + 1], MUL, ADD,
                            )
                    else:
                        nc.vector.tensor_scalar(
                            Hdst[:, jt, :], HRAW[:, jt, :],
                            A[:, j : j + 1], C[:, j : j + 1], MUL, ADD,
                        )

            def mlp_layer(li, kt, rhs, W, Hdst):
                """One layer: matmuls + relu + distributed BN into Hdst.
                Feature-tile groups [4,2,1,1]; each group's stats AllGather
                fires as soon as the group finishes, its consume path
                resolves under the following chains / next layer."""
                groups = [[0, 1, 2, 3], [4, 5], [6], [7]]
                splits = [False, False, True, True]

                Ss, ccs = [], []

                # group 0: k-outer over 4 concurrent full-batch psum chains
                g0 = groups[0]
                n0 = len(g0)
                S0 = spool.tile([128, 2 * n0], F32, tag="S_g0", name=f"S{li}_0")
                Ss.append(S0)
                pss = [
                    pspool.tile([128, BL], F32, tag="ps", name=f"ps_g{j}")
                    for j in range(n0)
                ]
                for k in range(kt):
                    for j in range(n0):
                        mm_pair(pss[j], W[:, k, j * 128 : (j + 1) * 128], rhs, k, kt)
                for j in range(n0):
                    relu_square(pss[j], j, S0, j, n0)
                ccs.append(bn_ag_start(li, 0, S0, 2 * n0))

                for gi, jts in enumerate(groups[1:], 1):
                    n = len(jts)
                    width = 4 * n if splits[gi] else 2 * n
                    S = spool.tile([128, width], F32, tag=f"S_g{gi}",
                                   name=f"S{li}_{gi}")
                    Ss.append(S)
                    for i, jt in enumerate(jts):
                        ps = pspool.tile(
                            [128, BL], F32, tag="ps", name=f"ps_s{gi}_{i}"
                        )
                        for k in range(kt):
                            mm_pair(
                                ps, W[:, k, jt * 128 : (jt + 1) * 128], rhs, k, kt
                            )
                        relu_square(ps, jt, S, i, n, split=splits[gi])
                    # finish the previous group (its AllGather has landed by
                    # now), then launch this group's AllGather
                    bn_finish(li, gi - 1, groups[gi - 1], ccs[gi - 1], Hdst,
                              split=splits[gi - 1])
                    ccs.append(bn_ag_start(li, gi, S, width))
                bn_finish(li, 3, groups[3], ccs[3], Hdst, split=splits[3])

            # ---- layers ----
            mlp_layer(0, KT_IN, XT, W1, H)
            mlp_layer(1, JT, H, W2, H2)
            mlp_layer(2, JT, H2, W3, H)

            # ---- layer 4 (no relu/BN): two half-batch chains so the first
            # half's copy + output DMA overlap the second half's matmuls ----
            for hi, (s, w) in enumerate(nhalves):
                ps4 = pspool.tile([CLSP, w], F32, tag="ps", name=f"ps4_{hi}")
                for k in range(JT):
                    nc.tensor.matmul(
                        ps4[:], W4[:, k, :], H[:, k, s : s + w],
                        start=(k == 0), stop=(k == JT - 1),
                    )
                OUTS = spool.tile([CLSP, w], F32, tag=f"OUTS{hi}")
                nc.scalar.copy(OUTS[:], ps4[:])
                nc.sync.dma_start(out_d[:, s : s + w], OUTS[:])

            # anchor the warmup AllGather into an unused out_d row (rows
            # >= CLS are never read back) so DCE keeps it; emitted last so
            # it never blocks the gpsimd queue mid-kernel.
            nc.gpsimd.dma_start(out_d[CLSP - 1 : CLSP, 0:1], wu_out[0:1, :])

    # NOTE: stripping the trigger-side DMA-completion wait from the BN
    # collectives was measured at 223.7us (-34us) but produces NaN — the
    # trigger's wait is the ONLY data-readiness gate for the mesh's own-copy
    # phase (its internal wait9 covers the copy it issues itself, not our
    # input DMA). The wait must stay; the collective path is ~15us floor.

    nc.compile()
    return nc


def _get_nc():
    if "nc" not in _CACHE:
        _CACHE["nc"] = _build()
    return _CACHE["nc"]


def _prep_inputs(x, W1, W2, W3, W4, g1, b1, g2, b2, g3, b3):
    x2 = np.asarray(x, dtype=np.float32).reshape(B, KIN)
    xt = np.ascontiguousarray(x2.T).astype(ml_dtypes.bfloat16)  # [3072, 8192]

    def pmajor(a):
        # [ktiles*128, free] -> [128, ktiles, free] (partition-major)
        kt = a.shape[0] // 128
        return np.ascontiguousarray(
            a.reshape(kt, 128, a.shape[1]).transpose(1, 0, 2)
        )

    def bin_t(w, pad=None):
        wb = np.where(np.asarray(w, dtype=np.float32) >= 0, 1.0, -1.0)
        wt = np.ascontiguousarray(wb.T).astype(ml_dtypes.bfloat16)  # [in, out]
        if pad is not None and wt.shape[1] < pad:
            wt = np.concatenate(
                [wt, np.zeros((wt.shape[0], pad - wt.shape[1]), wt.dtype)], axis=1
            )
        return pmajor(wt)

    w1t = bin_t(W1)            # [128, 24, 1024]
    w2t = bin_t(W2)            # [128, 8, 1024]
    w3t = bin_t(W3)
    w4t = bin_t(W4, pad=CLSP)  # [128, 8, 16]

    bnp = np.zeros((128, 6 * JT), dtype=np.float32)
    for l, p in enumerate([g1, b1, g2, b2, g3, b3]):
        pa = np.asarray(p, dtype=np.float32)
        for jt in range(JT):
            bnp[:, l * JT + jt] = pa[jt * 128 : (jt + 1) * 128]

    shared = {"w1t": w1t, "w2t": w2t, "w3t": w3t, "w4t": w4t, "bnp": bnp}
    in_maps = []
    for c in range(NCORES):
        m = dict(shared)
        m["xt"] = pmajor(np.ascontiguousarray(xt[:, c * BL : (c + 1) * BL]))
        in_maps.append(m)
    return in_maps


def _run(inputs, trace=False, trace_cores=None):
    nc = _get_nc()
    in_maps = _prep_inputs(**inputs)
    kw = {}
    if trace_cores is not None:
        kw["trace_cores"] = trace_cores
    res = bass_utils.run_bass_kernel_spmd(
        nc, in_maps, core_ids=list(range(NCORES)), trace=trace, **kw
    )
    out = np.empty((B, CLS), dtype=np.float32)
    for c in range(NCORES):
        out[c * BL : (c + 1) * BL, :] = res.results[c]["out"][:CLS, :].T
    return out, res


def kernel(**inputs):
    out, _ = _run(inputs, trace=False)
    return out


# revision 20
# speedup vs baseline: 1.0198x; 1.0198x over previous
"""Trainium2 Bass kernel for nn_BinaryLinear (binarized 4-layer MLP + BatchNorm).

Reference computation (fp32, jax):
    h = x.reshape(-1, 3072)
    h = relu(h @ sign(W1).T); h = BN(h, g1, b1)   # BN over full 8192 batch
    h = relu(h @ sign(W2).T); h = BN(h, g2, b2)
    h = relu(h @ sign(W3).T); h = BN(h, g3, b3)
    out = h @ sign(W4).T                          # [8192, 10]

Strategy (8 NeuronCores, data-parallel over batch):
  - Host: binarize weights to bf16 (+-1 exact), pack everything partition-
    major ([128, ktiles, free]), shard x over cores (1024 rows each).
  - Device (SPMD identical program): activations feature-major
    [feature_part, batch_free] in SBUF; K-tiled bf16 matmuls accumulate in
    fp32 PSUM. Per layer the 8 feature tiles run as groups [4,2,1,1]: the
    first four k-outer (consumes layer-1 input tiles in DMA arrival order),
    the rest sequential chains so relu/stats pipeline under the matmuls.
    The last two single-tile groups split relu/square per batch half so
    their stats leave before the chain tail retires.
  - Distributed BN: per-core (sum, sumsq) stats per group ride 4 small
    staggered AllGathers per layer; each group's gather + one-op core
    reduce + fused stats math + apply resolve under the following chains /
    the next layer's early k-tiles. A warmup AllGather absorbs the ~11us
    ncfw wake latency.
  - DMA-queue discipline: sync ring = XT feed, W2, all stats-out DMAs,
    final outputs; scalar ring = W1 feed, W3, W4, all gather DMAs; BNP on
    the GpSimd SWDGE queue. Queue contents are monotone in time, so no
    FIFO blocking at layer boundaries.
"""
import os
import sys

for _p in ("/opt/trn_rl_repo",):
    if os.path.isdir(_p) and _p not in sys.path:
        sys.path.insert(0, _p)

import numpy as np
import ml_dtypes

from concourse import bacc, tile, mybir
from concourse import bass_utils

NCORES = 8
B = 8192
BL = B // NCORES            # 1024 rows per core
HB = BL // 2                # 512-row batch half
KIN = 3072
KT_IN = KIN // 128          # 24 k-tiles for layer 1
HID = 1024
JT = HID // 128             # 8 feature tiles
CLS = 10
CLSP = 16                   # padded classes
EPS = 1e-5
BF16 = mybir.dt.bfloat16
F32 = mybir.dt.float32
ADD = mybir.AluOpType.add
SUB = mybir.AluOpType.subtract
MUL = mybir.AluOpType.mult
BYP = mybir.AluOpType.bypass
RELU = mybir.ActivationFunctionType.Relu
AXX = mybir.AxisListType.X
AXXY = mybir.AxisListType.XY

_CACHE = {}


def _build():
    nc = bacc.Bacc("TRN2", target_bir_lowering=False, debug=False, num_devices=NCORES)

    xt_d = nc.dram_tensor("xt", [128, KT_IN, BL], BF16, kind="ExternalInput")
    w1_d = nc.dram_tensor("w1t", [128, KT_IN, HID], BF16, kind="ExternalInput")
    w2_d = nc.dram_tensor("w2t", [128, JT, HID], BF16, kind="ExternalInput")
    w3_d = nc.dram_tensor("w3t", [128, JT, HID], BF16, kind="ExternalInput")
    w4_d = nc.dram_tensor("w4t", [128, JT, CLSP], BF16, kind="ExternalInput")
    bnp_d = nc.dram_tensor("bnp", [128, 6 * JT], F32, kind="ExternalInput")
    out_d = nc.dram_tensor("out", [CLSP, BL], F32, kind="ExternalOutput")

    nhalves = [(0, HB), (HB, HB)]
    bn_collectives = []  # BN stats AllGathers — trigger waits stripped below

    with tile.TileContext(nc) as tc:
        with (
            tc.tile_pool(name="weights", bufs=1) as wpool,
            tc.tile_pool(name="acts", bufs=1) as apool,
            tc.tile_pool(name="scratch", bufs=2) as scrpool,
            tc.tile_pool(name="stats", bufs=2) as spool,
            tc.tile_pool(name="psum", bufs=4, space="PSUM") as pspool,
            tc.tile_pool(name="dram", bufs=2, space="DRAM") as dpool,
        ):
            XT = wpool.tile([128, KT_IN, BL], BF16, tag="XT")
            W1 = wpool.tile([128, KT_IN, HID], BF16, tag="W1")
            W2 = wpool.tile([128, JT, HID], BF16, tag="W2")
            W3 = wpool.tile([128, JT, HID], BF16, tag="W3")
            W4 = wpool.tile([128, JT, CLSP], BF16, tag="W4")
            BNP = wpool.tile([128, 6 * JT], F32, tag="BNP")
            HRAW = apool.tile([128, JT, BL], BF16, tag="HRAW")
            H = apool.tile([128, JT, BL], BF16, tag="H")
            H2 = apool.tile([128, JT, BL], BF16, tag="H2")

            # Warmup collective: absorbs the ~11us ncfw wake latency off the
            # critical path. Input is an unwritten scratch buffer (contents
            # irrelevant); output anchored into an unused out_d row at the
            # end of the program so DCE keeps it.
            wu_in = dpool.tile([128, 1], F32, tag="wu_in")
            wu_out = dpool.tile([NCORES * 128, 1], F32, tag="wu_out")
            nc.gpsimd.collective_compute(
                "AllGather",
                BYP,
                replica_groups=[list(range(NCORES))],
                ins=[wu_in.opt()],
                outs=[wu_out.opt()],
            )

            # Input feed: XT on the Sync ring, W1 on the Scalar ring, in
            # growing chunks; W2/W3/W4 right after so nothing bulky is in
            # flight at the layer boundaries.
            nc.gpsimd.dma_start(BNP[:], bnp_d[:])
            feed = [2] * 12
            c = 0
            for w in feed:
                w = min(w, KT_IN - c)
                if w <= 0:
                    break
                nc.sync.dma_start(XT[:, c : c + w, :], xt_d[:, c : c + w, :])
                nc.scalar.dma_start(W1[:, c : c + w, :], w1_d[:, c : c + w, :])
                c += w
            nc.sync.dma_start(W2[:], w2_d[:])
            nc.scalar.dma_start(W3[:], w3_d[:])
            nc.scalar.dma_start(W4[:], w4_d[:])

            def mm_pair(ps, Wk, rhs, k, kt):
                for idx, (s, w) in enumerate(nhalves):
                    mi = nc.tensor.matmul(
                        ps[:, s : s + w],
                        Wk,
                        rhs[:, k, s : s + w],
                        start=(k == 0),
                        stop=(k == kt - 1),
                    )
                    if idx > 0:
                        # same stationary weights as the previous matmul:
                        # skip the redundant LDWEIGHTS
                        mi.ins.ldweights = False

            def relu_square(ps, jt, S, j, n, split=False):
                """relu: PSUM f32 -> HRAW bf16, accum = batch sum (free).
                sum(relu^2) via one VectorE scalar_tensor_tensor. split=True
                accumulates per batch-half (S slots: sumA,sqA,sumB,sqB) so
                the stats leave before the whole chain retires."""
                if not split:
                    nc.scalar.activation(
                        HRAW[:, jt, :], ps[:], RELU, accum_out=S[:, j : j + 1]
                    )
                    scr = scrpool.tile([128, BL], BF16, tag="scr")
                    nc.vector.scalar_tensor_tensor(
                        scr[:], HRAW[:, jt, :], 0.0, HRAW[:, jt, :], BYP, MUL,
                        accum_out=S[:, n + j : n + j + 1],
                    )
                else:
                    for h, (s, w) in enumerate(nhalves):
                        nc.scalar.activation(
                            HRAW[:, jt, s : s + w], ps[:, s : s + w], RELU,
                            accum_out=S[:, 2 * h : 2 * h + 1],
                        )
                        scr = scrpool.tile([128, HB], BF16, tag="scrh")
                        nc.vector.scalar_tensor_tensor(
                            scr[:], HRAW[:, jt, s : s + w], 0.0,
                            HRAW[:, jt, s : s + w], BYP, MUL,
                            accum_out=S[:, 2 * h + 1 : 2 * h + 2],
                        )

            def bn_ag_start(li, gi, S, width):
                """Stats out on the Sync HWDGE ring + AllGather trigger."""
                cc_in = dpool.tile([128, width], F32, tag=f"cc_in{gi}",
                                   name=f"cc_in_{li}_{gi}")
                cc_out = dpool.tile([NCORES * 128, width], F32, tag=f"cc_out{gi}",
                                    name=f"cc_out_{li}_{gi}")
                nc.sync.dma_start(cc_in[:], S[:])
                cc = nc.gpsimd.collective_compute(
                    "AllGather",
                    BYP,
                    replica_groups=[list(range(NCORES))],
                    ins=[cc_in.opt()],
                    outs=[cc_out.opt()],
                )
                bn_collectives.append(cc)
                return cc_out

            def bn_finish(li, gi, jts, cc_out, Hdst, split=False):
                """Gather stats, one-op core (+half) reduce, fused math,
                apply into Hdst."""
                n = len(jts)
                width = 4 * n if split else 2 * n
                GAT = spool.tile([128, NCORES, width], F32, tag=f"GAT{gi}",
                                 name=f"GAT_{li}_{gi}")
                # sync ring, NOT scalar: a gather blocked on its AllGather in
                # the Scalar HWDGE FIFO stalls the next relus (~11us/boundary,
                # v6 trace). Combined here with the smooth [2]*12 feed — each
                # fix alone was absorbed by the other hazard (v6/v7 runs).
                nc.sync.dma_start(
                    GAT[:], cc_out.opt().rearrange("(c p) s -> p c s", p=128)
                )
                SS = spool.tile([128, 2 * n], F32, tag=f"SS{gi}",
                                name=f"SS_{li}_{gi}")
                if split:
                    nc.vector.tensor_reduce(
                        SS[:], GAT[:].rearrange("p c (h s) -> p s h c", h=2),
                        axis=AXXY, op=ADD,
                    )
                else:
                    nc.vector.tensor_reduce(
                        SS[:], GAT[:].rearrange("p c s -> p s c"),
                        axis=AXX, op=ADD,
                    )
                MV = spool.tile([128, 2 * n], F32, tag=f"MV{gi}",
                                name=f"MV_{li}_{gi}")
                nc.vector.tensor_scalar_mul(MV[:], SS[:], 1.0 / B)
                mean = MV[:, 0:n]
                esq = MV[:, n : 2 * n]
                MSQ = spool.tile([128, n], F32, tag=f"MSQ{gi}",
                                 name=f"MSQ_{li}_{gi}")
                nc.vector.scalar_tensor_tensor(MSQ[:], mean, 0.0, mean, BYP, MUL)
                VAR = spool.tile([128, n], F32, tag=f"VAR{gi}",
                                 name=f"VAR_{li}_{gi}")
                nc.vector.scalar_tensor_tensor(VAR[:], esq, EPS, MSQ[:], ADD, SUB)
                RINV = spool.tile([128, n], F32, tag=f"RINV{gi}",
                                  name=f"RINV_{li}_{gi}")
                nc.vector.reciprocal(RINV[:], VAR[:])
                RSTD = spool.tile([128, n], F32, tag=f"RSTD{gi}",
                                  name=f"RSTD_{li}_{gi}")
                nc.scalar.sqrt(RSTD[:], RINV[:])
                g0 = (2 * li) * JT + jts[0]
                b0 = (2 * li + 1) * JT + jts[0]
                A = spool.tile([128, n], F32, tag=f"A{gi}", name=f"A_{li}_{gi}")
                nc.vector.tensor_tensor(A[:], RSTD[:], BNP[:, g0 : g0 + n], MUL)
                AM = spool.tile([128, n], F32, tag=f"AM{gi}", name=f"AM_{li}_{gi}")
                nc.vector.tensor_tensor(AM[:], A[:], mean, MUL)
                C = spool.tile([128, n], F32, tag=f"C{gi}", name=f"C_{li}_{gi}")
                nc.vector.tensor_tensor(C[:], BNP[:, b0 : b0 + n], AM[:], SUB)
                for j, jt in enumerate(jts):
                    if split:
                        for s, w in nhalves:
                            nc.vector.tensor_scalar(
                                Hdst[:, jt, s : s + w], HRAW[:, jt, s : s + w],
                                A[:, j : j + 1], C[:, j : j + 1], MUL, ADD,
                            )
                    else:
                        nc.vector.tensor_scalar(
                            Hdst[:, jt, :], HRAW[:, jt, :],
                            A[:, j : j + 1], C[:, j : j + 1], MUL, ADD,
                        )

            def mlp_layer(li, kt, rhs, W, Hdst):
                """One layer: matmuls + relu + distributed BN into Hdst.
                Feature-tile groups [4,2,1,1]; each group's stats AllGather
                fires as soon as the group finishes, its consume path
                resolves under the following chains / next layer."""
                groups = [[0, 1, 2, 3], [4, 5], [6], [7]]
                splits = [False, False, True, True]

                Ss, ccs = [], []

                # group 0: k-outer over 4 concurrent full-batch psum chains
                g0 = groups[0]
                n0 = len(g0)
                S0 = spool.tile([128, 2 * n0], F32, tag="S_g0", name=f"S{li}_0")
                Ss.append(S0)
                pss = [
                    pspool.tile([128, BL], F32, tag="ps", name=f"ps_g{j}")
                    for j in range(n0)
                ]
                for k in range(kt):
                    for j in range(n0):
                        mm_pair(pss[j], W[:, k, j * 128 : (j + 1) * 128], rhs, k, kt)
                for j in range(n0):
                    relu_square(pss[j], j, S0, j, n0)
                ccs.append(bn_ag_start(li, 0, S0, 2 * n0))

                for gi, jts in enumerate(groups[1:], 1):
                    n = len(jts)
                    width = 4 * n if splits[gi] else 2 * n
                    S = spool.tile([128, width], F32, tag=f"S_g{gi}",
                                   name=f"S{li}_{gi}")
                    Ss.append(S)
                    for i, jt in enumerate(jts):
                        ps = pspool.tile(
                            [128, BL], F32, tag="ps", name=f"ps_s{gi}_{i}"
                        )
                        for k in range(kt):
                            mm_pair(
                                ps, W[:, k, jt * 128 : (jt + 1) * 128], rhs, k, kt
                            )
                        relu_square(ps, jt, S, i, n, split=splits[gi])
                    # finish the previous group (its AllGather has landed by
                    # now), then launch this group's AllGather
                    bn_finish(li, gi - 1, groups[gi - 1], ccs[gi - 1], Hdst,
                              split=splits[gi - 1])
                    ccs.append(bn_ag_start(li, gi, S, width))
                bn_finish(li, 3, groups[3], ccs[3], Hdst, split=splits[3])

            # ---- layers ----
            mlp_layer(0, KT_IN, XT, W1, H)
            mlp_layer(1, JT, H, W2, H2)
            mlp_layer(2, JT, H2, W3, H)

            # ---- layer 4 (no relu/BN): two half-batch chains so the first
            # half's copy + output DMA overlap the second half's matmuls ----
            for hi, (s, w) in enumerate(nhalves):
                ps4 = pspool.tile([CLSP, w], F32, tag="ps", name=f"ps4_{hi}")
                for k in range(JT):
                    nc.tensor.matmul(
                        ps4[:], W4[:, k, :], H[:, k, s : s + w],
                        start=(k == 0), stop=(k == JT - 1),
                    )
                OUTS = spool.tile([CLSP, w], F32, tag=f"OUTS{hi}")
                nc.scalar.copy(OUTS[:], ps4[:])
                nc.sync.dma_start(out_d[:, s : s + w], OUTS[:])

            # anchor the warmup AllGather into an unused out_d row (rows
            # >= CLS are never read back) so DCE keeps it; emitted last so
            # it never blocks the gpsimd queue mid-kernel.
            nc.gpsimd.dma_start(out_d[CLSP - 1 : CLSP, 0:1], wu_out[0:1, :])

    # NOTE: stripping the trigger-side DMA-completion wait from the BN
    # collectives was measured at 223.7us (-34us) but produces NaN — the
    # trigger's wait is the ONLY data-readiness gate for the mesh's own-copy
    # phase (its internal wait9 covers the copy it issues itself, not our
    # input DMA). The wait must stay; the collective path is ~15us floor.

    nc.compile()
    return nc


def _get_nc():
    if "nc" not in _CACHE:
        _CACHE["nc"] = _build()
    return _CACHE["nc"]


def _prep_inputs(x, W1, W2, W3, W4, g1, b1, g2, b2, g3, b3):
    x2 = np.asarray(x, dtype=np.float32).reshape(B, KIN)
    xt = np.ascontiguousarray(x2.T).astype(ml_dtypes.bfloat16)  # [3072, 8192]

    def pmajor(a):
        # [ktiles*128, free] -> [128, ktiles, free] (partition-major)
        kt = a.shape[0] // 128
        return np.ascontiguousarray(
            a.reshape(kt, 128, a.shape[1]).transpose(1, 0, 2)
        )

    def bin_t(w, pad=None):
        wb = np.where(np.asarray(w, dtype=np.float32) >= 0, 1.0, -1.0)
        wt = np.ascontiguousarray(wb.T).astype(ml_dtypes.bfloat16)  # [in, out]
        if pad is not None and wt.shape[1] < pad:
            wt = np.concatenate(
                [wt, np.zeros((wt.shape[0], pad - wt.shape[1]), wt.dtype)], axis=1
            )
        return pmajor(wt)

    w1t = bin_t(W1)            # [128, 24, 1024]
    w2t = bin_t(W2)            # [128, 8, 1024]
    w3t = bin_t(W3)
    w4t = bin_t(W4, pad=CLSP)  # [128, 8, 16]

    bnp = np.zeros((128, 6 * JT), dtype=np.float32)
    for l, p in enumerate([g1, b1, g2, b2, g3, b3]):
        pa = np.asarray(p, dtype=np.float32)
        for jt in range(JT):
            bnp[:, l * JT + jt] = pa[jt * 128 : (jt + 1) * 128]

    shared = {"w1t": w1t, "w2t": w2t, "w3t": w3t, "w4t": w4t, "bnp": bnp}
    in_maps = []
    for c in range(NCORES):
        m = dict(shared)
        m["xt"] = pmajor(np.ascontiguousarray(xt[:, c * BL : (c + 1) * BL]))
        in_maps.append(m)
    return in_maps


def _run(inputs, trace=False, trace_cores=None):
    nc = _get_nc()
    in_maps = _prep_inputs(**inputs)
    kw = {}
    if trace_cores is not None:
        kw["trace_cores"] = trace_cores
    res = bass_utils.run_bass_kernel_spmd(
        nc, in_maps, core_ids=list(range(NCORES)), trace=trace, **kw
    )
    out = np.empty((B, CLS), dtype=np.float32)
    for c in range(NCORES):
        out[c * BL : (c + 1) * BL, :] = res.results[c]["out"][:CLS, :].T
    return out, res


def kernel(**inputs):
    out, _ = _run(inputs, trace=False)
    return out
